# revision 19
# baseline (speedup 1.0000x reference)
"""Bass/Trainium2 kernel for nn_Attn_81690277970335.

reference:  proj = enc @ W.T + b        [S, H]
            energies = proj @ hidden    [S]
            attn = softmax(energies)    [1, 1, S]

Key algebraic identity (exact in exact arithmetic):
            energies = enc @ (W.T @ hidden) + (b . hidden)
and softmax is invariant to the constant shift (b . hidden).  So on device we
compute v = W.T @ hidden once and then a single [S,H] @ [H] matvec over the
big tensor -> memory-bound streaming of enc.

Distribution: encoder_outputs sharded along seq across 8 cores; W, hidden
replicated (the first collective of an execution only reaches its mesh phase
~60us after its doorbell, so v must NOT depend on a collective).

DMA: W as 8 interleaved 0.5MB k-pieces first on both HWDGE rings (the PE
pipelines the v matmuls against their arrival; v is produced in two 512-col
halves so DVE dot products start on half 0 at ~26us), then enc pieces with
shrinking tails so only ~2 rows of compute trail the last DMA byte.

Dot products: per 4-row group, 3 rows on DVE (fused mult+accum
scalar_tensor_tensor with broadcast dummy out -> no product write-back,
~0.8us per [128,512] half), 1 row on GpSimd (plain mult, ~1.65us/half)
reduced by ACT (Copy+accum).  Each row is two half passes (cols 0:512 with
vA, 512:1024 with vB) accumulated into eA/eB, e = eA + eB at the end.

Softmax: per-partition max m_p / sum s_p; ONE AllGather (no dummy: a second
collective inherits the first's inter-rank completion spread, measured
+27us) of packed [2,128] (-m_p, s_p); every core redundantly combines all
8*128 pairs and rescales its exp(e - m_p) tile.
"""

import sys

sys.path.insert(0, "/opt/trn_rl_repo")

import numpy as np

import concourse.bass as bass
import concourse.mybir as mybir
import concourse.tile as tile
from concourse.bass_utils import run_bass_kernel_spmd

SEQ = 32768
HID = 1024
NCORES = 8
SHARD = SEQ // NCORES  # 4096
P = 128  # partitions
TW = SHARD // P  # 32 seq rows per partition
KCH = HID // P  # 8 contraction chunks for v
F32 = mybir.dt.float32
AL = mybir.AluOpType
ACT = mybir.ActivationFunctionType

# enc pieces: (ring, t_start, t_end).  ring 0 = sync, ring 1 = scalar.
# Landing order pairs pieces across rings; tails shrink so little compute
# trails the last byte.
ENC_PIECES = [
    (0, 0, 2),
    (1, 2, 4),
    (0, 4, 8),
    (1, 8, 12),
    (0, 12, 16),
    (1, 16, 20),
    (0, 20, 23),
    (1, 23, 26),
    (0, 26, 28),
    (1, 28, 30),
    (0, 30, 31),
    (1, 31, 32),
]

_CACHE = {}


def _split_multiwaits(nc):
    """This container's walrus build accepts at most ONE sync-wait per
    instruction; Tile emits several.  Hoist extra waits onto single-wait
    NoOps inserted just before the instruction on the same engine queue
    (engines and DGE-issuing sequencers are in-order, so semantics hold)."""
    import bass_rust

    cnt = 0
    for f in nc.m.functions:
        for bb in f.blocks:
            il = bb.instructions
            i = 0
            while i < len(il):
                inst = il[i]
                si = inst.sync_info
                if si is not None and si.on_wait and len(si.on_wait) > 1:
                    waits = list(si.on_wait)
                    keep, extra = waits[-1], waits[:-1]
                    for j, w in enumerate(extra):
                        nop = mybir.InstNoOp(
                            name=f"{inst.name}-w{j}", ins=[], outs=[]
                        )
                        nop.engine = inst.engine
                        nop.sync_info = bass_rust.SyncInfo(
                            on_wait=[w], on_update=[]
                        )
                        il.insert(i, nop)
                        i += 1
                        cnt += 1
                    inst.sync_info = bass_rust.SyncInfo(
                        on_wait=[keep], on_update=list(si.on_update or [])
                    )
                i += 1
    return cnt


def _build_nc():
    nc = bass.Bass(num_devices=NCORES)

    enc = nc.dram_tensor("enc", [SHARD, HID], F32, kind="ExternalInput")
    # full W, host-restaged so o-chunk k, row p = W[k*128+p, :]:
    # wt[p, k, h] = W[k*128+p, h]
    wt = nc.dram_tensor("wt", [P, KCH, HID], F32, kind="ExternalInput")
    # aux: [128, 8 + 128 + 128]: hid_pk | ident | ones
    AUXW = KCH + P + P
    aux = nc.dram_tensor("aux", [P, AUXW], F32, kind="ExternalInput")
    out = nc.dram_tensor("attn", [SHARD], F32, kind="ExternalOutput")

    dummy_in = nc.dram_tensor("dummy_in", [1, 1], F32)
    dummy_out = nc.dram_tensor("dummy_out", [NCORES, 1], F32, addr_space="Shared")
    cc_in = nc.dram_tensor("cc_in", [2, P], F32)
    cc_out = nc.dram_tensor("cc_out", [2 * NCORES, P], F32, addr_space="Shared")

    # seq row s of the shard lives at (partition p, column t): s = p*TW + t
    enc3 = enc.rearrange("(p t) h -> p t h", t=TW)  # [128, 32, 1024]
    out_v = out.rearrange("(p t) -> p t", t=TW)  # [128, 32]

    rings = [nc.sync, nc.scalar]

    with tile.TileContext(nc) as tc:
        with (
            tc.tile_pool(name="wpool", bufs=1) as wpool,
            tc.tile_pool(name="encp", bufs=1) as encp,
            tc.tile_pool(name="jg", bufs=3) as jgp,
            tc.tile_pool(name="small", bufs=1) as small,
            tc.tile_pool(name="ps_v", bufs=1, space="PSUM") as ps_v,
            tc.tile_pool(name="ps_c", bufs=1, space="PSUM") as ps_c,
        ):
            # ---- aux (tiny) then 4 W k-pair pieces on both rings ----------
            # (no dummy collective: the CC stream init runs lazily in the
            # background and completes at 60-78us per core; with compute
            # triggering the real AllGather by ~50-65us the mesh rides the
            # init completion directly, and a dummy mesh would only queue
            # the real one behind the slowest core's dummy)
            aux_sb = wpool.tile([P, AUXW], F32, tag="aux")
            nc.sync.dma_start(out=aux_sb[:], in_=aux[:])
            w_sb = []
            for g in range(KCH // 2):
                wg = wpool.tile([P, 2, HID], F32, tag=f"w{g}", name=f"w{g}")
                rings[g % 2].dma_start(
                    out=wg[:], in_=wt[:, 2 * g : 2 * g + 2, :]
                )
                w_sb.append(wg)

            hid_pk = aux_sb[:, 0:KCH]  # [128, 8] hidden o-chunks
            ident = aux_sb[:, KCH : KCH + P]  # [128, 128] identity
            ones_row = aux_sb[0:1, KCH + P : KCH + 2 * P]  # [1, 128] of 1.0

            # ---- enc pieces (each tile distinct; whole shard fits SBUF) ---
            enc_ts = []
            for ring, ta, tb in ENC_PIECES:
                t = encp.tile([P, tb - ta, HID], F32, tag=f"enc{ta}", name=f"enc{ta}")
                rings[ring].dma_start(out=t[:], in_=enc3[:, ta:tb, :])
                enc_ts.append(t)

            # ---- v = W.T @ hidden, replicated on all partitions, halves ---
            # stationary = hidden o-chunk broadcast into all 128 PE columns
            # -> result lands replicated.  n=0 half first so DVE starts early.
            vb_ps = [
                ps_v.tile([P, 512], F32, tag=f"vb{n}", name=f"vb_ps{n}")
                for n in range(2)
            ]
            for n in range(2):
                for k in range(KCH):
                    nc.tensor.matmul(
                        vb_ps[n][:],
                        hid_pk[:, k : k + 1].broadcast_to([P, P]),
                        w_sb[k // 2][:, k % 2, n * 512 : (n + 1) * 512],
                        start=(k == 0),
                        stop=(k == KCH - 1),
                    )
            # PE finishes both v halves by ~35us, before the first enc
            # piece lands -> one full-width vb, full-row dot products.
            vb = small.tile([P, HID], F32, tag="vb")
            nc.vector.tensor_copy(vb[:, 0:512], vb_ps[0][:])
            nc.vector.tensor_copy(vb[:, 512:1024], vb_ps[1][:])

            # ---- energies: E[p, t] = enc_row . v, one fused pass per row --
            # rows with t%4==3 on GpSimd (mult) + ACT (Copy+accum reduce);
            # the rest on DVE (fused mult+accum, broadcast dummy out).
            e_sb = small.tile([P, TW], F32, tag="e")
            jd = small.tile([P, 1], F32, tag="jd_dummy")

            def dve_row(enc_t, u, t_idx):
                nc.vector.scalar_tensor_tensor(
                    out=jd.broadcast_to([P, HID]),
                    in0=enc_t[:, u, :],
                    scalar=1.0,
                    in1=vb[:],
                    op0=AL.mult,
                    op1=AL.mult,
                    accum_out=e_sb[:, t_idx : t_idx + 1],
                )

            def gp_row(enc_t, u, t_idx):
                jg = jgp.tile([P, HID], F32, name="jg")
                nc.gpsimd.tensor_tensor(
                    out=jg[:], in0=enc_t[:, u, :], in1=vb[:], op=AL.mult
                )
                nc.scalar.activation(
                    jg[:],
                    jg[:],
                    ACT.Copy,
                    accum_out=e_sb[:, t_idx : t_idx + 1],
                )

            for pi, (ring, ta, tb) in enumerate(ENC_PIECES):
                for u in range(tb - ta):
                    t_idx = ta + u
                    if t_idx % 4 == 3 or t_idx == 2:
                        gp_row(enc_ts[pi], u, t_idx)
                    else:
                        dve_row(enc_ts[pi], u, t_idx)

            # ---- local per-partition softmax stats ------------------------
            # ms[:,0] = -m_p (negated row max), ms[:,1] = s_p = sum exp(e-m_p)
            ms = small.tile([P, 2], F32, tag="ms")
            m_sb = small.tile([P, 1], F32, tag="m")
            nc.vector.tensor_reduce(
                m_sb[:], e_sb[:], axis=mybir.AxisListType.X, op=AL.max
            )
            nc.vector.tensor_scalar(
                out=ms[:, 0:1],
                in0=m_sb[:],
                scalar1=-1.0,
                scalar2=None,
                op0=AL.mult,
            )
            eexp = small.tile([P, TW], F32, tag="eexp")
            nc.scalar.activation(
                eexp[:],
                e_sb[:],
                ACT.Exp,
                bias=ms[:, 0:1],
                accum_out=ms[:, 1:2],
            )

            # ---- exchange per-partition stats: [2,128] x 8 cores ----------
            tr_ps = ps_c.tile([2, P], F32, tag="tr", name="tr_ps")
            nc.tensor.transpose(tr_ps[:], ms[:], ident)
            cc_sb = small.tile([2, P], F32, tag="ccs")
            nc.vector.tensor_copy(cc_sb[:], tr_ps[:])
            nc.sync.dma_start(out=cc_in[:], in_=cc_sb[:])
            nc.gpsimd.collective_compute(
                "AllGather",
                AL.bypass,
                replica_groups=[list(range(NCORES))],
                ins=[cc_in.ap().opt()],
                outs=[cc_out.ap().opt()],
            )
            ag_sb = small.tile([1, 2 * NCORES * P], F32, tag="ag")
            nc.sync.dma_start(
                out=ag_sb[:], in_=cc_out.rearrange("a b -> (a b)")
            )
            ag4 = ag_sb[:].rearrange(
                "p (r two h) -> p r two h", r=NCORES, two=2
            )
            nm_all = ag4[:, :, 0, :]  # [1, 8, 128] of -m_rp
            s_all = ag4[:, :, 1, :]  # [1, 8, 128] of s_rp

            # g2[:,0] = gnm = min(-m) = -M ; g2[:,1] = 1/gsum
            g2 = small.tile([1, 2], F32, tag="g2")
            nc.vector.tensor_reduce(
                g2[:, 0:1], nm_all, axis=mybir.AxisListType.XY, op=AL.min
            )
            # edifs = exp(-(nm - gnm)) = exp(m_rp - M)  (bias folds the sub)
            edifs = small.tile([1, NCORES, P], F32, tag="edifs")
            nc.scalar.activation(
                edifs[:], nm_all, ACT.Exp, bias=g2[:, 0:1], scale=-1.0
            )
            # gsum = sum s_rp * exp(m_rp - M), fused mult+accum
            jd2 = small.tile([1, 1], F32, tag="jd2_dummy")
            gsum = small.tile([1, 1], F32, tag="gsum")
            nc.vector.scalar_tensor_tensor(
                out=jd2.broadcast_to(edifs[:].shape),
                in0=edifs[:],
                scalar=1.0,
                in1=s_all,
                op0=AL.mult,
                op1=AL.mult,
                accum_out=gsum[:],
            )
            nc.vector.reciprocal(g2[:, 1:2], gsum[:])

            # ---- broadcast (gnm, 1/gsum) to all partitions, rescale -------
            bc_ps = ps_c.tile([P, 2], F32, tag="bc", name="bc_ps")
            nc.tensor.matmul(bc_ps[:], ones_row, g2[:], start=True, stop=True)
            d_p = small.tile([P, 1], F32, tag="dp")
            nc.vector.tensor_tensor(
                out=d_p[:], in0=ms[:, 0:1], in1=bc_ps[:, 0:1], op=AL.subtract
            )
            # f0 = exp(-(nm_p - gnm)) = exp(m_p - M)
            f0 = small.tile([P, 1], F32, tag="f0")
            nc.scalar.activation(f0[:], d_p[:], ACT.Exp, scale=-1.0)
            f = small.tile([P, 1], F32, tag="f")
            nc.vector.tensor_tensor(
                out=f[:], in0=f0[:], in1=bc_ps[:, 1:2], op=AL.mult
            )

            # ---- attn = eexp * f, store -----------------------------------
            attn_sb = small.tile([P, TW], F32, tag="attn")
            nc.scalar.mul(attn_sb[:], eexp[:], f[:])
            nc.sync.dma_start(out=out_v, in_=attn_sb[:])

    _split_multiwaits(nc)
    return nc


def _get_nc():
    if "nc" not in _CACHE:
        _CACHE["nc"] = _build_nc()
    return _CACHE["nc"]


def _prep_in_maps(hidden, encoder_outputs, W, b):
    hidden = np.ascontiguousarray(np.asarray(hidden, dtype=np.float32))
    enc = np.ascontiguousarray(np.asarray(encoder_outputs, dtype=np.float32))
    W = np.ascontiguousarray(np.asarray(W, dtype=np.float32))
    # wt[p, k, h] = W[k*128+p, h]
    wt = np.ascontiguousarray(W.reshape(KCH, P, HID).transpose(1, 0, 2))
    hid_pk = hidden.reshape(KCH, P).T  # [128, 8]
    ident = np.eye(P, dtype=np.float32)
    ones = np.ones((P, P), dtype=np.float32)
    auxc = np.ascontiguousarray(
        np.concatenate([hid_pk, ident, ones], axis=1), dtype=np.float32
    )
    in_maps = []
    for c in range(NCORES):
        in_maps.append(
            {
                "enc": enc[c * SHARD : (c + 1) * SHARD],
                "wt": wt,
                "aux": auxc,
            }
        )
    return in_maps


def _ensure_ntff_hook():
    """Register the axon NTFF profile hook that this deployment's antenv
    package is missing, so trace=True yields a real HW profile."""
    import sys as _sys
    import types

    if "antenv.axon_hooks" in _sys.modules:
        return
    mod = types.ModuleType("antenv.axon_hooks")
    holder = [None]
    mod.set_axon_ntff_profile_hook = lambda h: holder.__setitem__(0, h)
    mod.get_axon_ntff_profile_hook = lambda: holder[0]
    _sys.modules["antenv.axon_hooks"] = mod
    import antenv

    antenv.axon_hooks = mod
    try:
        if "/root/.axon_site" not in _sys.path:
            _sys.path.insert(0, "/root/.axon_site")
        from trn_agent_boot.trn_boot import _ntff_profile_via_ctypes

        hook = _ntff_profile_via_ctypes("/opt/axon/libaxon_pjrt.so")
        if hook is not None:
            mod.set_axon_ntff_profile_hook(hook)
    except Exception as e:  # degrade to no tracing
        print(f"ntff hook registration failed: {e}", file=_sys.stderr)
    # artifact upload needs no external bucket for local profiling
    from concourse import bass_utils as _bu

    _bu.upload_artifacts = lambda tmpdir: tmpdir


def run(hidden, encoder_outputs, W, b, trace=False, **trace_kw):
    if trace:
        _ensure_ntff_hook()
    nc = _get_nc()
    in_maps = _prep_in_maps(hidden, encoder_outputs, W, b)
    res = run_bass_kernel_spmd(
        nc, in_maps, list(range(NCORES)), trace=trace, **trace_kw
    )
    shards = [np.asarray(res.results[c]["attn"]) for c in range(NCORES)]
    full = np.concatenate(shards).astype(np.float32)
    return full[None, None, :], res


def kernel(hidden, encoder_outputs, W, b):
    out, _ = run(hidden, encoder_outputs, W, b, trace=False)
    return out


# revision 21
# speedup vs baseline: 1.1968x; 1.1968x over previous
"""Bass/Trainium2 kernel for nn_Attn_81690277970335.

reference:  proj = enc @ W.T + b        [S, H]
            energies = proj @ hidden    [S]
            attn = softmax(energies)    [1, 1, S]

Key algebraic identity (exact in exact arithmetic):
            energies = enc @ (W.T @ hidden) + (b . hidden)
and softmax is invariant to the constant shift (b . hidden).  So on device we
compute v = W.T @ hidden once and then a single [S,H] @ [H] matvec over the
big tensor -> memory-bound streaming of enc.

Distribution: encoder_outputs sharded along seq across 8 cores; W, hidden
replicated (the first collective of an execution only reaches its mesh phase
~60us after its doorbell, so v must NOT depend on a collective).

DMA: W as 8 interleaved 0.5MB k-pieces first on both HWDGE rings (the PE
pipelines the v matmuls against their arrival; v is produced in two 512-col
halves so DVE dot products start on half 0 at ~26us), then enc pieces with
shrinking tails so only ~2 rows of compute trail the last DMA byte.

Dot products: per 4-row group, 3 rows on DVE (fused mult+accum
scalar_tensor_tensor with broadcast dummy out -> no product write-back,
~0.8us per [128,512] half), 1 row on GpSimd (plain mult, ~1.65us/half)
reduced by ACT (Copy+accum).  Each row is two half passes (cols 0:512 with
vA, 512:1024 with vB) accumulated into eA/eB, e = eA + eB at the end.

Softmax: per-partition max m_p / sum s_p; ONE AllGather (no dummy: a second
collective inherits the first's inter-rank completion spread, measured
+27us) of packed [2,128] (-m_p, s_p); every core redundantly combines all
8*128 pairs and rescales its exp(e - m_p) tile.
"""

import sys

sys.path.insert(0, "/opt/trn_rl_repo")

import numpy as np

import concourse.bass as bass
import concourse.mybir as mybir
import concourse.tile as tile
from concourse.bass_utils import run_bass_kernel_spmd

SEQ = 32768
HID = 1024
NCORES = 8
SHARD = SEQ // NCORES  # 4096
P = 128  # partitions
TW = SHARD // P  # 32 seq rows per partition
KCH = HID // P  # 8 contraction chunks for v
F32 = mybir.dt.float32
AL = mybir.AluOpType
ACT = mybir.ActivationFunctionType

# enc pieces: (ring, t_start, t_end).  ring 0 = sync, ring 1 = scalar.
# Landing order pairs pieces across rings; tails shrink so little compute
# trails the last byte.
ENC_PIECES = [
    (0, 0, 2),
    (1, 2, 4),
    (0, 4, 8),
    (1, 8, 12),
    (0, 12, 16),
    (1, 16, 20),
    (0, 20, 23),
    (1, 23, 26),
    (0, 26, 28),
    (1, 28, 30),
    (0, 30, 31),
    (1, 31, 32),
]

_CACHE = {}


def _split_multiwaits(nc):
    """This container's walrus build accepts at most ONE sync-wait per
    instruction; Tile emits several.  Hoist extra waits onto single-wait
    NoOps inserted just before the instruction on the same engine queue
    (engines and DGE-issuing sequencers are in-order, so semantics hold)."""
    import bass_rust

    cnt = 0
    for f in nc.m.functions:
        for bb in f.blocks:
            il = bb.instructions
            i = 0
            while i < len(il):
                inst = il[i]
                si = inst.sync_info
                if si is not None and si.on_wait and len(si.on_wait) > 1:
                    waits = list(si.on_wait)
                    keep, extra = waits[-1], waits[:-1]
                    for j, w in enumerate(extra):
                        nop = mybir.InstNoOp(
                            name=f"{inst.name}-w{j}", ins=[], outs=[]
                        )
                        nop.engine = inst.engine
                        nop.sync_info = bass_rust.SyncInfo(
                            on_wait=[w], on_update=[]
                        )
                        il.insert(i, nop)
                        i += 1
                        cnt += 1
                    inst.sync_info = bass_rust.SyncInfo(
                        on_wait=[keep], on_update=list(si.on_update or [])
                    )
                i += 1
    return cnt


def _build_nc():
    nc = bass.Bass(num_devices=NCORES)

    enc = nc.dram_tensor("enc", [SHARD, HID], F32, kind="ExternalInput")
    # full W, host-restaged so o-chunk k, row p = W[k*128+p, :]:
    # wt[p, k, h] = W[k*128+p, h]
    wt = nc.dram_tensor("wt", [P, KCH, HID], F32, kind="ExternalInput")
    # aux: [128, 8 + 128 + 128]: hid_pk | ident | ones
    AUXW = KCH + P + P
    aux = nc.dram_tensor("aux", [P, AUXW], F32, kind="ExternalInput")
    out = nc.dram_tensor("attn", [SHARD], F32, kind="ExternalOutput")

    dummy_in = nc.dram_tensor("dummy_in", [1, 1], F32)
    dummy_out = nc.dram_tensor("dummy_out", [NCORES, 1], F32, addr_space="Shared")
    cc_in = nc.dram_tensor("cc_in", [2, P], F32)
    cc_out = nc.dram_tensor("cc_out", [2 * NCORES, P], F32, addr_space="Shared")

    # seq row s of the shard lives at (partition p, column t): s = p*TW + t
    enc3 = enc.rearrange("(p t) h -> p t h", t=TW)  # [128, 32, 1024]
    out_v = out.rearrange("(p t) -> p t", t=TW)  # [128, 32]

    rings = [nc.sync, nc.scalar]

    with tile.TileContext(nc) as tc:
        with (
            tc.tile_pool(name="wpool", bufs=1) as wpool,
            tc.tile_pool(name="encp", bufs=1) as encp,
            tc.tile_pool(name="jg", bufs=3) as jgp,
            tc.tile_pool(name="small", bufs=1) as small,
            tc.tile_pool(name="ps_v", bufs=1, space="PSUM") as ps_v,
            tc.tile_pool(name="ps_c", bufs=1, space="PSUM") as ps_c,
        ):
            # ---- aux (tiny) then 4 W k-pair pieces on both rings ----------
            # (no dummy collective: the CC stream init runs lazily in the
            # background and completes at 60-78us per core; with compute
            # triggering the real AllGather by ~50-65us the mesh rides the
            # init completion directly, and a dummy mesh would only queue
            # the real one behind the slowest core's dummy)
            aux_sb = wpool.tile([P, AUXW], F32, tag="aux")
            nc.sync.dma_start(out=aux_sb[:], in_=aux[:])
            # W in column halves: wa* (cols 0:512 of all k) first so the
            # n=0 matmul group and vA complete ~13us before full-W would
            wa_sb, wb_sb = [], []
            for r in range(2):
                wg = wpool.tile([P, 4, 512], F32, tag=f"wa{r}", name=f"wa{r}")
                rings[r].dma_start(
                    out=wg[:], in_=wt[:, 4 * r : 4 * r + 4, 0:512]
                )
                wa_sb.append(wg)

            hid_pk = aux_sb[:, 0:KCH]  # [128, 8] hidden o-chunks
            ident = aux_sb[:, KCH : KCH + P]  # [128, 128] identity
            ones_row = aux_sb[0:1, KCH + P : KCH + 2 * P]  # [1, 128] of 1.0

            # ---- enc pieces (each tile distinct; whole shard fits SBUF);
            # wb* (W cols 512:1024) slot in after the first enc pair -------
            enc_ts = []
            for pi, (ring, ta, tb) in enumerate(ENC_PIECES):
                t = encp.tile([P, tb - ta, HID], F32, tag=f"enc{ta}", name=f"enc{ta}")
                rings[ring].dma_start(out=t[:], in_=enc3[:, ta:tb, :])
                enc_ts.append(t)
                if pi == 1:
                    for r in range(2):
                        wg = wpool.tile(
                            [P, 4, 512], F32, tag=f"wb{r}", name=f"wb{r}"
                        )
                        rings[r].dma_start(
                            out=wg[:], in_=wt[:, 4 * r : 4 * r + 4, 512:1024]
                        )
                        wb_sb.append(wg)

            # ---- v = W.T @ hidden, replicated on all partitions, halves ---
            # stationary = hidden o-chunk broadcast into all 128 PE columns
            # -> result lands replicated.  n=0 half first so DVE starts early.
            vb_ps = [
                ps_v.tile([P, 512], F32, tag=f"vb{n}", name=f"vb_ps{n}")
                for n in range(2)
            ]
            for n in range(2):
                src = wa_sb if n == 0 else wb_sb
                for k in range(KCH):
                    nc.tensor.matmul(
                        vb_ps[n][:],
                        hid_pk[:, k : k + 1].broadcast_to([P, P]),
                        src[k // 4][:, k % 4, :],
                        start=(k == 0),
                        stop=(k == KCH - 1),
                    )
            # vA completes ~21us (cols 0:512 of W land first), vB ~35us;
            # every row is two half passes so DVE/GP start on vA early.
            vA = small.tile([P, 512], F32, tag="vA")
            vB = small.tile([P, 512], F32, tag="vB")
            vh = [vA, vB]

            eh = [
                small.tile([P, TW], F32, tag="eA", name="eA"),
                small.tile([P, TW], F32, tag="eB", name="eB"),
            ]
            jd = small.tile([P, 1], F32, tag="jd_dummy")

            def dve_half(pi, u, t_idx, n):
                nc.vector.scalar_tensor_tensor(
                    out=jd.broadcast_to([P, 512]),
                    in0=enc_ts[pi][:, u, n * 512 : (n + 1) * 512],
                    scalar=1.0,
                    in1=vh[n][:],
                    op0=AL.mult,
                    op1=AL.mult,
                    accum_out=eh[n][:, t_idx : t_idx + 1],
                )

            def gp_half(pi, u, t_idx, n):
                jg = jgp.tile([P, 512], F32, name="jg")
                nc.gpsimd.tensor_tensor(
                    out=jg[:],
                    in0=enc_ts[pi][:, u, n * 512 : (n + 1) * 512],
                    in1=vh[n][:],
                    op=AL.mult,
                )
                nc.scalar.activation(
                    jg[:],
                    jg[:],
                    ACT.Copy,
                    accum_out=eh[n][:, t_idx : t_idx + 1],
                )

            def piece_rows(pi, n, eng):
                ring, ta, tb = ENC_PIECES[pi]
                for u in range(tb - ta):
                    t_idx = ta + u
                    gp = t_idx % 4 == 3
                    if gp and eng == "gp":
                        gp_half(pi, u, t_idx, n)
                    elif not gp and eng == "dve":
                        dve_half(pi, u, t_idx, n)

            # DVE queue: vA copy, A-halves of pieces 0-1, vB copy, their
            # B-halves, then per piece A,B.  GP mirrors on its own queue.
            nc.vector.tensor_copy(vA[:], vb_ps[0][:])
            piece_rows(0, 0, "dve")
            piece_rows(1, 0, "dve")
            nc.vector.tensor_copy(vB[:], vb_ps[1][:])
            piece_rows(0, 1, "dve")
            piece_rows(1, 1, "dve")
            for pi in (0, 1):
                piece_rows(pi, 0, "gp")
                piece_rows(pi, 1, "gp")
            for pi in range(2, len(ENC_PIECES)):
                piece_rows(pi, 0, "dve")
                piece_rows(pi, 1, "dve")
                piece_rows(pi, 0, "gp")
                piece_rows(pi, 1, "gp")

            e_sb = small.tile([P, TW], F32, tag="e")
            nc.vector.tensor_tensor(
                out=e_sb[:], in0=eh[0][:], in1=eh[1][:], op=AL.add
            )

            # ---- local per-partition softmax stats ------------------------
            # ms[:,0] = -m_p (negated row max), ms[:,1] = s_p = sum exp(e-m_p)
            ms = small.tile([P, 2], F32, tag="ms")
            m_sb = small.tile([P, 1], F32, tag="m")
            nc.vector.tensor_reduce(
                m_sb[:], e_sb[:], axis=mybir.AxisListType.X, op=AL.max
            )
            nc.vector.tensor_scalar(
                out=ms[:, 0:1],
                in0=m_sb[:],
                scalar1=-1.0,
                scalar2=None,
                op0=AL.mult,
            )
            eexp = small.tile([P, TW], F32, tag="eexp")
            nc.scalar.activation(
                eexp[:],
                e_sb[:],
                ACT.Exp,
                bias=ms[:, 0:1],
                accum_out=ms[:, 1:2],
            )

            # ---- exchange per-partition stats: [2,128] x 8 cores ----------
            tr_ps = ps_c.tile([2, P], F32, tag="tr", name="tr_ps")
            nc.tensor.transpose(tr_ps[:], ms[:], ident)
            cc_sb = small.tile([2, P], F32, tag="ccs")
            nc.vector.tensor_copy(cc_sb[:], tr_ps[:])
            nc.sync.dma_start(out=cc_in[:], in_=cc_sb[:])
            nc.gpsimd.collective_compute(
                "AllGather",
                AL.bypass,
                replica_groups=[list(range(NCORES))],
                ins=[cc_in.ap().opt()],
                outs=[cc_out.ap().opt()],
            )
            ag_sb = small.tile([1, 2 * NCORES * P], F32, tag="ag")
            nc.sync.dma_start(
                out=ag_sb[:], in_=cc_out.rearrange("a b -> (a b)")
            )
            ag4 = ag_sb[:].rearrange(
                "p (r two h) -> p r two h", r=NCORES, two=2
            )
            nm_all = ag4[:, :, 0, :]  # [1, 8, 128] of -m_rp
            s_all = ag4[:, :, 1, :]  # [1, 8, 128] of s_rp

            # g2[:,0] = gnm = min(-m) = -M ; g2[:,1] = 1/gsum
            g2 = small.tile([1, 2], F32, tag="g2")
            nc.vector.tensor_reduce(
                g2[:, 0:1], nm_all, axis=mybir.AxisListType.XY, op=AL.min
            )
            # edifs = exp(-(nm - gnm)) = exp(m_rp - M)  (bias folds the sub)
            edifs = small.tile([1, NCORES, P], F32, tag="edifs")
            nc.scalar.activation(
                edifs[:], nm_all, ACT.Exp, bias=g2[:, 0:1], scale=-1.0
            )
            # gsum = sum s_rp * exp(m_rp - M), fused mult+accum
            jd2 = small.tile([1, 1], F32, tag="jd2_dummy")
            gsum = small.tile([1, 1], F32, tag="gsum")
            nc.vector.scalar_tensor_tensor(
                out=jd2.broadcast_to(edifs[:].shape),
                in0=edifs[:],
                scalar=1.0,
                in1=s_all,
                op0=AL.mult,
                op1=AL.mult,
                accum_out=gsum[:],
            )
            nc.vector.reciprocal(g2[:, 1:2], gsum[:])

            # ---- broadcast (gnm, 1/gsum) to all partitions, rescale -------
            bc_ps = ps_c.tile([P, 2], F32, tag="bc", name="bc_ps")
            nc.tensor.matmul(bc_ps[:], ones_row, g2[:], start=True, stop=True)
            d_p = small.tile([P, 1], F32, tag="dp")
            nc.vector.tensor_tensor(
                out=d_p[:], in0=ms[:, 0:1], in1=bc_ps[:, 0:1], op=AL.subtract
            )
            # f0 = exp(-(nm_p - gnm)) = exp(m_p - M)
            f0 = small.tile([P, 1], F32, tag="f0")
            nc.scalar.activation(f0[:], d_p[:], ACT.Exp, scale=-1.0)
            f = small.tile([P, 1], F32, tag="f")
            nc.vector.tensor_tensor(
                out=f[:], in0=f0[:], in1=bc_ps[:, 1:2], op=AL.mult
            )

            # ---- attn = eexp * f, store -----------------------------------
            attn_sb = small.tile([P, TW], F32, tag="attn")
            nc.scalar.mul(attn_sb[:], eexp[:], f[:])
            nc.sync.dma_start(out=out_v, in_=attn_sb[:])

    _split_multiwaits(nc)
    return nc


def _get_nc():
    if "nc" not in _CACHE:
        _CACHE["nc"] = _build_nc()
    return _CACHE["nc"]


def _prep_in_maps(hidden, encoder_outputs, W, b):
    hidden = np.ascontiguousarray(np.asarray(hidden, dtype=np.float32))
    enc = np.ascontiguousarray(np.asarray(encoder_outputs, dtype=np.float32))
    W = np.ascontiguousarray(np.asarray(W, dtype=np.float32))
    # wt[p, k, h] = W[k*128+p, h]
    wt = np.ascontiguousarray(W.reshape(KCH, P, HID).transpose(1, 0, 2))
    hid_pk = hidden.reshape(KCH, P).T  # [128, 8]
    ident = np.eye(P, dtype=np.float32)
    ones = np.ones((P, P), dtype=np.float32)
    auxc = np.ascontiguousarray(
        np.concatenate([hid_pk, ident, ones], axis=1), dtype=np.float32
    )
    in_maps = []
    for c in range(NCORES):
        in_maps.append(
            {
                "enc": enc[c * SHARD : (c + 1) * SHARD],
                "wt": wt,
                "aux": auxc,
            }
        )
    return in_maps


def _ensure_ntff_hook():
    """Register the axon NTFF profile hook that this deployment's antenv
    package is missing, so trace=True yields a real HW profile."""
    import sys as _sys
    import types

    if "antenv.axon_hooks" in _sys.modules:
        return
    mod = types.ModuleType("antenv.axon_hooks")
    holder = [None]
    mod.set_axon_ntff_profile_hook = lambda h: holder.__setitem__(0, h)
    mod.get_axon_ntff_profile_hook = lambda: holder[0]
    _sys.modules["antenv.axon_hooks"] = mod
    import antenv

    antenv.axon_hooks = mod
    try:
        if "/root/.axon_site" not in _sys.path:
            _sys.path.insert(0, "/root/.axon_site")
        from trn_agent_boot.trn_boot import _ntff_profile_via_ctypes

        hook = _ntff_profile_via_ctypes("/opt/axon/libaxon_pjrt.so")
        if hook is not None:
            mod.set_axon_ntff_profile_hook(hook)
    except Exception as e:  # degrade to no tracing
        print(f"ntff hook registration failed: {e}", file=_sys.stderr)
    # artifact upload needs no external bucket for local profiling
    from concourse import bass_utils as _bu

    _bu.upload_artifacts = lambda tmpdir: tmpdir


def run(hidden, encoder_outputs, W, b, trace=False, **trace_kw):
    if trace:
        _ensure_ntff_hook()
    nc = _get_nc()
    in_maps = _prep_in_maps(hidden, encoder_outputs, W, b)
    res = run_bass_kernel_spmd(
        nc, in_maps, list(range(NCORES)), trace=trace, **trace_kw
    )
    shards = [np.asarray(res.results[c]["attn"]) for c in range(NCORES)]
    full = np.concatenate(shards).astype(np.float32)
    return full[None, None, :], res


def kernel(hidden, encoder_outputs, W, b):
    out, _ = run(hidden, encoder_outputs, W, b, trace=False)
    return out
